# revision 1
# baseline (speedup 1.0000x reference)
"""Category-specific MLP (MoE-style routing) on 8 Trainium2 NeuronCores.

Host-routed expert parallelism with cat-grouped "slots":
  - The host groups the 64 samples by cat_id and packs them into 8 cores x
    `profile` slots (profile uniform across cores so one SPMD program
    serves all 8; e.g. [3,2,2,1] samples per slot), so all of a slot's
    tokens share one weight bank. Groups are split across slots/cores when
    the histogram demands it (costs an extra bank load, still correct).
  - Per (slot, layer) the weight bank is one SBUF tile [128, 8*1024] bf16;
    matmuls slice it per (k-tile, m-tile). Tokens beyond 512 are processed
    as extra N<=512 matmuls chained on the same loaded weights
    (`dedupe_ldweights` removes the redundant LDWEIGHTS; the remaining
    loads are folded into self-loading matmuls by `fuse_ldweights`),
    amortizing the serial ~53ns weight-load cost per 128x128 tile.
  - Activations live transposed [D, tok]; each layer computes
    out_T = W_l.T @ h_T via matmul(lhsT=W tile, rhs=h_T), so layers chain
    on the tensor engine with no transposes. bf16 on chip (fp32 PSUM
    accumulate), bf16 output DMA, converted to f32 on the host.
  - DMA queues: SP (HWDGE) carries x loads, output stores and the startup
    per-k weight slices; GPSIMD (SWDGE, otherwise-idle Pool engine)
    carries the bulk weight banks, so the ACT engine runs only Silu
    epilogues and never stalls behind DMA descriptor generation. The bias
    table is pre-permuted on the host so its load is one contiguous DMA.
"""

import numpy as np
from contextlib import ExitStack

import ml_dtypes

import concourse.bass as bass
import concourse.mybir as mybir
import concourse.tile as tile
from concourse import bacc
from concourse.bass_utils import run_bass_kernel_spmd
from concourse.tile_rust import add_dep_helper

P = 128
D = 1024
KT = D // P          # 8 k-tiles per dim
TOK = 256            # tokens per sample
S = 8                # samples per core
L = 4
NCORES = 8
MAXN = 512           # PSUM bank limit (fp32 columns per matmul)

ACT_DT = mybir.dt.bfloat16
W_DT = mybir.dt.bfloat16
ACT_NP = ml_dtypes.bfloat16
W_NP = ml_dtypes.bfloat16

LAST_RESULT = None
_PROGRAM_CACHE = {}

SLOT_ORDER = "desc"


# ---------------------------------------------------------------- planning

def _partitions(n, maxp):
    """Partitions of n into parts <= maxp, descending."""
    if n == 0:
        yield ()
        return
    for first in range(min(n, maxp), 0, -1):
        for rest in _partitions(n - first, first):
            yield (first,) + rest


def _feasible(profile, sizes):
    """Greedy: fill 8 copies of each slot size (desc) from cat blocks,
    splitting blocks when needed. 'Exact size first, else largest'."""
    rem = sorted(sizes, reverse=True)
    slots = []
    for pos, g in enumerate(profile):
        slots += [(g, pos)] * NCORES
    slots.sort(key=lambda t: -t[0])
    for g, pos in slots:
        if g in rem:
            rem.remove(g)
            continue
        if not rem or rem[0] < g:
            return False
        rem[0] -= g
        rem.sort(reverse=True)
        while rem and rem[-1] == 0:
            rem.pop()
    return True


def plan(cat_ids):
    """Returns (profile, cores): profile = tuple of slot sample-counts;
    cores[c] = list of per-slot sample-index lists, each from one cat."""
    cat_ids = np.asarray(cat_ids).astype(np.int64)
    by_cat = {}
    for i, c in enumerate(cat_ids.tolist()):
        by_cat.setdefault(c, []).append(i)
    blocks = sorted(by_cat.values(), key=len, reverse=True)
    sizes = [len(b) for b in blocks]

    profiles = sorted(set(_partitions(S, S)), key=lambda p: (len(p), -p[0]))
    chosen = None
    for prof in profiles:
        if _feasible(prof, sizes):
            chosen = prof
            break
    assert chosen is not None  # [1]*8 is always feasible

    rem = [list(b) for b in blocks]
    per_pos = {pos: [] for pos in range(len(chosen))}
    slots = []
    for pos, g in enumerate(chosen):
        slots += [(g, pos)] * NCORES
    slots.sort(key=lambda t: -t[0])
    for g, pos in slots:
        exact = [b for b in rem if len(b) == g]
        src = exact[0] if exact else max(rem, key=len)
        assert len(src) >= g
        per_pos[pos].append(src[:g])
        del src[:g]
    cores = []
    for c in range(NCORES):
        cores.append([per_pos[pos][c] for pos in range(len(chosen))])
    return chosen, cores


def _slot_order(profile):
    """Largest slot first: the startup k-outer streaming paces best with
    more tokens per k-slice, and the smallest slot last shortens the
    output tail."""
    if SLOT_ORDER == "desc":
        return sorted(range(len(profile)), key=lambda i: -profile[i])
    idx = sorted(range(len(profile)), key=lambda i: profile[i])
    first = idx[0]
    rest = sorted([i for i in idx if i != first],
                  key=lambda i: -profile[i])
    return [first] + rest


def _subs(tok):
    out = []
    off = 0
    while off < tok:
        n = min(MAXN, tok - off)
        out.append((off, n))
        off += n
    return out


# ------------------------------------------------------- LDWEIGHTS passes

def dedupe_ldweights(nc):
    """Remove InstLdweights whose weights AP is identical to the previous
    PE weight load with only matmuls in between: the array still holds
    those weights, so the reload is redundant. The matmul keeps the weight
    tile in its `ins`, so tile lifetimes are unaffected; waits/updates on
    a removed load merge into the following matmul. Tile names are unique
    per ring allocation, so an address reused for new data never compares
    equal."""
    n_removed = 0
    for bb in nc.m.functions[0].blocks:
        insts = bb.instructions
        last_ap = None
        drop = []
        for i in range(len(insts)):
            inst = insts[i]
            t = type(inst).__name__
            if t == "InstLdweights":
                ap = str(inst.ins[0])
                if ap == last_ap:
                    drop.append(i)
                else:
                    last_ap = ap
            elif t in ("InstMatmult", "InstEventSemaphore"):
                pass
            elif getattr(inst, "engine", None) == mybir.EngineType.PE:
                last_ap = None
        for i in reversed(drop):
            inst = insts[i]
            si = inst.sync_info
            if si is not None and (si.on_wait or si.on_update):
                for j in range(i + 1, len(insts)):
                    if type(insts[j]).__name__ == "InstMatmult":
                        tgt = insts[j]
                        if tgt.sync_info is None:
                            tgt.sync_info = mybir.SyncInfo(
                                on_wait=list(si.on_wait),
                                on_update=list(si.on_update))
                        else:
                            tgt.sync_info.on_wait.extend(si.on_wait)
                            tgt.sync_info.on_update.extend(si.on_update)
                        break
            del insts[i]
            n_removed += 1
    return n_removed


def pipeline_ldweights(nc):
    """Move each InstLdweights one matmul earlier in the PE stream, so the
    load for weight tile k+1 issues while tile k's last matmul is still
    streaming. Only correct if the PE weight path is double-buffered —
    verified by the rel-err check."""
    for bb in nc.m.functions[0].blocks:
        insts = bb.instructions
        pe_idx = [i for i, inst in enumerate(insts)
                  if type(inst).__name__ in ("InstLdweights", "InstMatmult")]
        pe_seq = [insts[i] for i in pe_idx]
        new_seq = []
        for inst in pe_seq:
            if (type(inst).__name__ == "InstLdweights" and new_seq
                    and type(new_seq[-1]).__name__ == "InstMatmult"):
                new_seq.insert(len(new_seq) - 1, inst)
            else:
                new_seq.append(inst)
        for i, inst in zip(pe_idx, new_seq):
            insts[i] = inst


def fuse_ldweights(nc):
    """Fold each remaining InstLdweights into its matmul (ldweights=True)
    so walrus generates the weight load itself (fast-weight-load path).
    Waits move onto the matmul."""
    for bb in nc.m.functions[0].blocks:
        insts = bb.instructions
        drop = []
        pending = None
        for i in range(len(insts)):
            inst = insts[i]
            t = type(inst).__name__
            if t == "InstLdweights":
                pending = (i, inst)
            elif t == "InstMatmult" and pending is not None:
                li, ld = pending
                if str(ld.ins[0]) == str(inst.ins[1]):
                    inst.ldweights = True
                    si = ld.sync_info
                    if si is not None and (si.on_wait or si.on_update):
                        if inst.sync_info is None:
                            inst.sync_info = mybir.SyncInfo(
                                on_wait=list(si.on_wait),
                                on_update=list(si.on_update))
                        else:
                            inst.sync_info.on_wait.extend(si.on_wait)
                            inst.sync_info.on_update.extend(si.on_update)
                    drop.append(li)
                pending = None
        for i in reversed(drop):
            del insts[i]


# ---------------------------------------------------------------- program

def build_program(profile, reps=1, mode="full", chain=True, selfload=True,
                  wb_bufs=5, h_gens=3, o_gens=1, ps_bufs=8,
                  x0_split=(1, 1, 2, 4), tail_stream=True, ldw_ahead=False):
    """One SPMD program for all 8 cores: len(profile) slots x 4 layers.

    reps>1 wraps the body in a hardware loop (wall-clock slope timing in
    the test harness; grading uses reps=1). mode: "full" (graded),
    "dma_only" / "compute_only" for bottleneck attribution.
    """
    sorder = _slot_order(profile)
    toks = [profile[i] * TOK for i in sorder]
    NS = len(toks)
    offs = np.concatenate([[0], np.cumsum(toks)])

    nc = bacc.Bacc("TRN2", target_bir_lowering=False, debug=False,
                   num_devices=NCORES)
    xT = nc.dram_tensor("xT", [D, S * TOK], ACT_DT, kind="ExternalInput")
    wg = nc.dram_tensor("wg", [NS, L, D, D], W_DT, kind="ExternalInput")
    # bias pre-permuted on host to [P, L*NS*KT] so the load is contiguous
    bg = nc.dram_tensor("bg", [P, L * NS * KT], mybir.dt.float32,
                        kind="ExternalInput")
    outT = nc.dram_tensor("outT", [D, S * TOK], ACT_DT,
                          kind="ExternalOutput")

    xv = xT.ap().rearrange("(k p) n -> p k n", p=P)
    ov = outT.ap().rearrange("(k p) n -> p k n", p=P)
    bv = bg.ap()

    silu = mybir.ActivationFunctionType.Silu

    with tile.TileContext(nc) as tc, ExitStack() as ctx:
        wpool = ctx.enter_context(tc.tile_pool(name="w", bufs=wb_bufs))
        hpool = ctx.enter_context(tc.tile_pool(name="h", bufs=h_gens))
        opool = ctx.enter_context(tc.tile_pool(name="o", bufs=o_gens))
        ppool = ctx.enter_context(
            tc.tile_pool(name="ps", bufs=ps_bufs, space="PSUM"))
        cpool = ctx.enter_context(tc.tile_pool(name="c", bufs=1))

        btile = cpool.tile([P, L * NS * KT], mybir.dt.float32)
        nc.scalar.dma_start(btile[:], bv[:, :])

        ms = list(reversed(range(KT)))
        ks = list(range(1, KT)) + [0]

        def body(_iv=None):
            frozen_w = {}
            for s in range(NS):
                tok, off = toks[s], int(offs[s])
                subs = _subs(tok)
                # one [P, 8*tok] activation tile per layer; column k*tok+i
                # holds token i of k-slice k
                hb = hpool.tile([P, KT * tok], ACT_DT, tag=f"A{tok}",
                                name="hin")
                if s == 0 and mode != "dma_only":
                    # split the x load so the k-outer matmuls stream behind
                    k0 = 0
                    for nk in x0_split:
                        nc.gpsimd.dma_start(
                            hb[:, k0 * tok:(k0 + nk) * tok],
                            xv[:, k0:k0 + nk, off:off + tok])
                        k0 += nk
                else:
                    nc.sync.dma_start(hb[:, :], xv[:, :, off:off + tok])
                for l in range(L):
                    # ---- weight bank DMA
                    if mode == "compute_only" and l in frozen_w:
                        wt = frozen_w[l]
                    else:
                        wt = wpool.tile([P, KT * D], W_DT, tag="wb",
                                        name=f"w{s}_{l}")
                        wsrc = wg.ap()[s, l].rearrange("(k p) m -> p k m",
                                                       p=P)
                        if s == 0 and l == 0 and mode != "dma_only":
                            # per-k slices on SP: fast issue, PE streams
                            # behind them
                            for k in range(KT):
                                nc.sync.dma_start(
                                    wt[:, k * D:(k + 1) * D], wsrc[:, k, :])
                        else:
                            h = KT // 2
                            nc.gpsimd.dma_start(wt[:, :h * D],
                                                wsrc[:, :h, :])
                            nc.gpsimd.dma_start(wt[:, h * D:],
                                                wsrc[:, h:, :])
                        if mode == "compute_only":
                            frozen_w[l] = wt

                    if mode == "dma_only":
                        continue  # hb keeps the x data; out-DMA reads it

                    last = l == L - 1
                    if last:
                        ob = opool.tile([P, KT * tok], ACT_DT,
                                        tag=f"O{tok}", name="ob")
                    else:
                        ob = hpool.tile([P, KT * tok], ACT_DT,
                                        tag=f"A{tok}", name="hu")

                    def epilogue(m, si, ps):
                        soff, n = subs[si]
                        col = (l * NS + s) * KT + m
                        dst = ob[:, m * tok + soff:m * tok + soff + n]
                        if last:
                            # bias-add on DVE, bf16 out; ACT keeps running
                            # only Silu (no activation-table switches)
                            nc.vector.tensor_scalar_add(
                                dst, ps[:, :n], btile[:, col:col + 1])
                        else:
                            nc.scalar.activation(dst, ps[:, :n], silu,
                                                 bias=btile[:, col:col + 1])

                    if s == 0 and l == 0:
                        # k-outer: stream behind the first DMAs. PSUM holds
                        # 8 banks, so process m in groups of 8//nsub.
                        gsize = max(1, KT // len(subs))
                        for g0 in range(0, KT, gsize):
                            mg = ms[g0:g0 + gsize]
                            pss = {m: [ppool.tile([P, MAXN],
                                                  mybir.dt.float32,
                                                  tag="ps",
                                                  name=f"ps{m}_{si}")
                                       for si in range(len(subs))]
                                   for m in mg}
                            for j in range(KT):
                                for m in mg:
                                    for si, (soff, n) in enumerate(subs):
                                        nc.tensor.matmul(
                                            pss[m][si][:, :n],
                                            wt[:, j * D + m * P:
                                               j * D + (m + 1) * P],
                                            hb[:, j * tok + soff:
                                               j * tok + soff + n],
                                            start=(j == 0),
                                            stop=(j == KT - 1))
                            for m in mg:
                                for si in range(len(subs)):
                                    epilogue(m, si, pss[m][si])
                    else:
                        # m-outer, k rotated so the next layer consumes the
                        # previous layer's last-produced tile last
                        for m in ms:
                            pss = [ppool.tile([P, MAXN], mybir.dt.float32,
                                              tag="ps", name=f"psm{si}")
                                   for si in range(len(subs))]
                            for j, k in enumerate(ks):
                                lead = None
                                for si, (soff, n) in enumerate(subs):
                                    mm = nc.tensor.matmul(
                                        pss[si][:, :n],
                                        wt[:, k * D + m * P:
                                           k * D + (m + 1) * P],
                                        hb[:, k * tok + soff:
                                           k * tok + soff + n],
                                        start=(j == 0), stop=(j == KT - 1))
                                    if si == 0:
                                        lead = mm
                                    elif chain:
                                        # keep the pair adjacent on PE so
                                        # the LDWEIGHTS dedupe can fire
                                        add_dep_helper(
                                            mm.ins, lead.ins, sync=False,
                                            reason="ldw chain")
                            for si in range(len(subs)):
                                epilogue(m, si, pss[si])
                    hb = ob
                # ---- output DMA: one store for the whole slot; for the
                # final slot, stream per-m stores in production order so
                # the tail is one m-tile, not the whole slot
                if tail_stream and s == NS - 1 and mode != "dma_only":
                    for m in ms:
                        nc.sync.dma_start(ov[:, m, off:off + tok],
                                          hb[:, m * tok:(m + 1) * tok])
                else:
                    nc.sync.dma_start(ov[:, :, off:off + tok], hb[:, :])

        if reps == 1:
            body()
        else:
            with tc.For_i(0, reps, 1) as iv:
                body(iv)
    if chain:
        dedupe_ldweights(nc)
    if ldw_ahead:
        pipeline_ldweights(nc)
    elif selfload:
        fuse_ldweights(nc)
    nc.compile()
    return nc


# ---------------------------------------------------------------- host glue

def prepare_in_maps(x, cat_ids, Ws, bs, profile, cores):
    x = np.asarray(x)
    cat_ids = np.asarray(cat_ids).astype(np.int64)
    sorder = _slot_order(profile)
    NS = len(profile)
    in_maps = []
    for c in range(NCORES):
        slots = [cores[c][pos] for pos in sorder]
        samp = [i for sl in slots for i in sl]
        xs = np.asarray(x[samp], dtype=np.float32)
        xTc = np.ascontiguousarray(xs.reshape(len(samp) * TOK, D).T)
        cats = [int(cat_ids[sl[0]]) for sl in slots]
        wgc = np.stack([np.stack([Ws[l][cat] for l in range(L)])
                        for cat in cats])
        # [L, NS, D] -> [P, L*NS*KT] with element [p, (l*NS+s)*KT+m]
        # = b_l[cat_s][m*P+p]
        bgc = np.stack([np.stack([bs[l][cat] for cat in cats])
                        for l in range(L)])
        bgc = bgc.reshape(L, NS, KT, P).transpose(3, 0, 1, 2).reshape(
            P, L * NS * KT)
        in_maps.append({
            "xT": xTc.astype(ACT_NP),
            "wg": np.ascontiguousarray(wgc).astype(W_NP),
            "bg": np.ascontiguousarray(bgc).astype(np.float32),
        })
    return in_maps


def finish_output(results, profile, cores, B):
    sorder = _slot_order(profile)
    out = np.empty((B, TOK, D), np.float32)
    for c in range(NCORES):
        slots = [cores[c][pos] for pos in sorder]
        samp = [i for sl in slots for i in sl]
        outTc = np.asarray(results[c]["outT"], dtype=np.float32)
        out[samp] = outTc.T.reshape(len(samp), TOK, D)
    return out


def kernel(x, cat_ids, W1, b1, W2, b2, W3, b3, W4, b4):
    global LAST_RESULT
    cat_ids = np.asarray(cat_ids).astype(np.int64)
    Ws = [np.asarray(w, dtype=np.float32) for w in (W1, W2, W3, W4)]
    bs = [np.asarray(b, dtype=np.float32) for b in (b1, b2, b3, b4)]
    x = np.asarray(x, dtype=np.float32)
    B = x.shape[0]

    profile, cores = plan(cat_ids)
    in_maps = prepare_in_maps(x, cat_ids, Ws, bs, profile, cores)

    if profile not in _PROGRAM_CACHE:
        _PROGRAM_CACHE[profile] = build_program(profile)
    nc = _PROGRAM_CACHE[profile]

    res = run_bass_kernel_spmd(nc, in_maps, list(range(NCORES)))
    LAST_RESULT = res
    return finish_output(res.results, profile, cores, B)



# revision 9
# speedup vs baseline: 1.0614x; 1.0614x over previous
"""Category-specific MLP (MoE-style routing) on 8 Trainium2 NeuronCores.

Host-routed expert parallelism with cat-grouped "slots":
  - The host groups the 64 samples by cat_id and packs them into 8 cores x
    `profile` slots (profile uniform across cores so one SPMD program
    serves all 8; e.g. [3,2,2,1] samples per slot), so all of a slot's
    tokens share one weight bank. Groups are split across slots/cores when
    the histogram demands it (costs an extra bank load, still correct).
  - Per (slot, layer) the weight bank is one SBUF tile [128, 8*1024] bf16;
    matmuls slice it per (k-tile, m-tile). Tokens beyond 512 are processed
    as extra N<=512 matmuls chained on the same loaded weights
    (`dedupe_ldweights` removes the redundant LDWEIGHTS; the remaining
    loads are folded into self-loading matmuls by `fuse_ldweights`),
    amortizing the serial ~53ns weight-load cost per 128x128 tile.
  - Activations live transposed [D, tok]; each layer computes
    out_T = W_l.T @ h_T via matmul(lhsT=W tile, rhs=h_T), so layers chain
    on the tensor engine with no transposes. bf16 on chip (fp32 PSUM
    accumulate), bf16 output DMA, converted to f32 on the host.
  - DMA queues: SP (HWDGE) carries x loads, output stores and the startup
    per-k weight slices; GPSIMD (SWDGE, otherwise-idle Pool engine)
    carries the bulk weight banks, so the ACT engine runs only Silu
    epilogues and never stalls behind DMA descriptor generation. The bias
    table is pre-permuted on the host so its load is one contiguous DMA.

Optimization findings (second session, kept as negative results; the
defaults below reproduce the original baseline stream exactly):
  - The kernel sits at the machine's effective roofline. bf16 matmul
    floor is 218.5us/core at the nominal 0.4167ns/col; measured
    compute_only (weights frozen in SBUF) is ~278us, full ~286-300us
    depending on session. Removing 842 of the 1036 weight loads
    (one_weight=True diagnostic) does NOT speed it up, and doubling the
    matmul instruction count (maxn=256) does not slow it down - so
    neither LDWEIGHTS nor per-instruction overhead explains the gap; the
    effective PE rate is simply below nominal (clock/HAM duty).
  - fp8 (e4m3, DoubleRow, 2x rate) fails numerics: ANY single-fp8
    operand gives rel-err ~5.2e-2 vs the 2e-2 gate (quantization noise
    scales with sqrt(K) exactly like the signal). hi/lo split-precision
    passes numerically (3e-3) but needs >= 1.5 DoubleRow passes per
    k-tile = 0.85x cycles at best - not worth the risk/complexity.
  - Measured worse or neutral on interleaved A/B: selfload=False
    (standalone LDWEIGHTS), ldw_ahead, w_split (weights on both DMA
    queues), ps_bufs=4, s0_gpsimd. Keep the defaults.
"""

import numpy as np
from contextlib import ExitStack

import ml_dtypes

import concourse.bass as bass
import concourse.mybir as mybir
import concourse.tile as tile
from concourse import bacc
from concourse.bass_utils import run_bass_kernel_spmd
from concourse.tile_rust import add_dep_helper

P = 128
D = 1024
KT = D // P          # 8 k-tiles per dim
TOK = 256            # tokens per sample
S = 8                # samples per core
L = 4
NCORES = 8
MAXN = 512           # PSUM bank limit (fp32 columns per matmul)

ACT_DT = mybir.dt.bfloat16
W_DT = mybir.dt.bfloat16
ACT_NP = ml_dtypes.bfloat16
W_NP = ml_dtypes.bfloat16

LAST_RESULT = None
_PROGRAM_CACHE = {}

SLOT_ORDER = "desc"


# ---------------------------------------------------------------- planning

def _partitions(n, maxp):
    """Partitions of n into parts <= maxp, descending."""
    if n == 0:
        yield ()
        return
    for first in range(min(n, maxp), 0, -1):
        for rest in _partitions(n - first, first):
            yield (first,) + rest


def _feasible(profile, sizes):
    """Greedy: fill 8 copies of each slot size (desc) from cat blocks,
    splitting blocks when needed. 'Exact size first, else largest'."""
    rem = sorted(sizes, reverse=True)
    slots = []
    for pos, g in enumerate(profile):
        slots += [(g, pos)] * NCORES
    slots.sort(key=lambda t: -t[0])
    for g, pos in slots:
        if g in rem:
            rem.remove(g)
            continue
        if not rem or rem[0] < g:
            return False
        rem[0] -= g
        rem.sort(reverse=True)
        while rem and rem[-1] == 0:
            rem.pop()
    return True


def plan(cat_ids):
    """Returns (profile, cores): profile = tuple of slot sample-counts;
    cores[c] = list of per-slot sample-index lists, each from one cat."""
    cat_ids = np.asarray(cat_ids).astype(np.int64)
    by_cat = {}
    for i, c in enumerate(cat_ids.tolist()):
        by_cat.setdefault(c, []).append(i)
    blocks = sorted(by_cat.values(), key=len, reverse=True)
    sizes = [len(b) for b in blocks]

    profiles = sorted(set(_partitions(S, S)), key=lambda p: (len(p), -p[0]))
    chosen = None
    for prof in profiles:
        if _feasible(prof, sizes):
            chosen = prof
            break
    assert chosen is not None  # [1]*8 is always feasible

    rem = [list(b) for b in blocks]
    per_pos = {pos: [] for pos in range(len(chosen))}
    slots = []
    for pos, g in enumerate(chosen):
        slots += [(g, pos)] * NCORES
    slots.sort(key=lambda t: -t[0])
    for g, pos in slots:
        exact = [b for b in rem if len(b) == g]
        src = exact[0] if exact else max(rem, key=len)
        assert len(src) >= g
        per_pos[pos].append(src[:g])
        del src[:g]
    cores = []
    for c in range(NCORES):
        cores.append([per_pos[pos][c] for pos in range(len(chosen))])
    return chosen, cores


def _slot_order(profile):
    """Largest slot first: the startup k-outer streaming paces best with
    more tokens per k-slice, and the smallest slot last shortens the
    output tail."""
    if SLOT_ORDER == "desc":
        return sorted(range(len(profile)), key=lambda i: -profile[i])
    idx = sorted(range(len(profile)), key=lambda i: profile[i])
    first = idx[0]
    rest = sorted([i for i in idx if i != first],
                  key=lambda i: -profile[i])
    return [first] + rest


def _subs(tok, maxn=MAXN):
    out = []
    off = 0
    while off < tok:
        n = min(maxn, tok - off)
        out.append((off, n))
        off += n
    return out


# ------------------------------------------------------- LDWEIGHTS passes

def dedupe_ldweights(nc):
    """Remove InstLdweights whose weights AP is identical to the previous
    PE weight load with only matmuls in between: the array still holds
    those weights, so the reload is redundant. The matmul keeps the weight
    tile in its `ins`, so tile lifetimes are unaffected; waits/updates on
    a removed load merge into the following matmul. Tile names are unique
    per ring allocation, so an address reused for new data never compares
    equal."""
    n_removed = 0
    for bb in nc.m.functions[0].blocks:
        insts = bb.instructions
        last_ap = None
        drop = []
        for i in range(len(insts)):
            inst = insts[i]
            t = type(inst).__name__
            if t == "InstLdweights":
                ap = str(inst.ins[0])
                if ap == last_ap:
                    drop.append(i)
                else:
                    last_ap = ap
            elif t in ("InstMatmult", "InstEventSemaphore"):
                pass
            elif getattr(inst, "engine", None) == mybir.EngineType.PE:
                last_ap = None
        for i in reversed(drop):
            inst = insts[i]
            si = inst.sync_info
            if si is not None and (si.on_wait or si.on_update):
                for j in range(i + 1, len(insts)):
                    if type(insts[j]).__name__ == "InstMatmult":
                        tgt = insts[j]
                        if tgt.sync_info is None:
                            tgt.sync_info = mybir.SyncInfo(
                                on_wait=list(si.on_wait),
                                on_update=list(si.on_update))
                        else:
                            tgt.sync_info.on_wait.extend(si.on_wait)
                            tgt.sync_info.on_update.extend(si.on_update)
                        break
            del insts[i]
            n_removed += 1
    return n_removed


def pipeline_ldweights(nc):
    """Move each InstLdweights one matmul earlier in the PE stream, so the
    load for weight tile k+1 issues while tile k's last matmul is still
    streaming. Only correct if the PE weight path is double-buffered —
    verified by the rel-err check."""
    for bb in nc.m.functions[0].blocks:
        insts = bb.instructions
        pe_idx = [i for i, inst in enumerate(insts)
                  if type(inst).__name__ in ("InstLdweights", "InstMatmult")]
        pe_seq = [insts[i] for i in pe_idx]
        new_seq = []
        for inst in pe_seq:
            if (type(inst).__name__ == "InstLdweights" and new_seq
                    and type(new_seq[-1]).__name__ == "InstMatmult"):
                new_seq.insert(len(new_seq) - 1, inst)
            else:
                new_seq.append(inst)
        for i, inst in zip(pe_idx, new_seq):
            insts[i] = inst


def fuse_ldweights(nc):
    """Fold each remaining InstLdweights into its matmul (ldweights=True)
    so walrus generates the weight load itself (fast-weight-load path).
    Waits move onto the matmul."""
    for bb in nc.m.functions[0].blocks:
        insts = bb.instructions
        drop = []
        pending = None
        for i in range(len(insts)):
            inst = insts[i]
            t = type(inst).__name__
            if t == "InstLdweights":
                pending = (i, inst)
            elif t == "InstMatmult" and pending is not None:
                li, ld = pending
                if str(ld.ins[0]) == str(inst.ins[1]):
                    inst.ldweights = True
                    si = ld.sync_info
                    if si is not None and (si.on_wait or si.on_update):
                        if inst.sync_info is None:
                            inst.sync_info = mybir.SyncInfo(
                                on_wait=list(si.on_wait),
                                on_update=list(si.on_update))
                        else:
                            inst.sync_info.on_wait.extend(si.on_wait)
                            inst.sync_info.on_update.extend(si.on_update)
                    drop.append(li)
                pending = None
        for i in reversed(drop):
            del insts[i]


# ---------------------------------------------------------------- program

def build_program(profile, reps=1, mode="full", chain=True, selfload=True,
                  wb_bufs=5, h_gens=3, o_gens=1, ps_bufs=8,
                  x0_split=(1, 1, 2, 4), tail_stream=True, ldw_ahead=False,
                  one_weight=False, w_split=False, s0_gpsimd=False,
                  maxn=MAXN):
    """One SPMD program for all 8 cores: len(profile) slots x 4 layers.

    reps>1 wraps the body in a hardware loop (wall-clock slope timing in
    the test harness; grading uses reps=1). mode: "full" (graded),
    "dma_only" / "compute_only" for bottleneck attribution.
    """
    sorder = _slot_order(profile)
    toks = [profile[i] * TOK for i in sorder]
    NS = len(toks)
    offs = np.concatenate([[0], np.cumsum(toks)])

    nc = bacc.Bacc("TRN2", target_bir_lowering=False, debug=False,
                   num_devices=NCORES)
    xT = nc.dram_tensor("xT", [D, S * TOK], ACT_DT, kind="ExternalInput")
    wg = nc.dram_tensor("wg", [NS, L, D, D], W_DT, kind="ExternalInput")
    # bias pre-permuted on host to [P, L*NS*KT] so the load is contiguous
    bg = nc.dram_tensor("bg", [P, L * NS * KT], mybir.dt.float32,
                        kind="ExternalInput")
    outT = nc.dram_tensor("outT", [D, S * TOK], ACT_DT,
                          kind="ExternalOutput")

    xv = xT.ap().rearrange("(k p) n -> p k n", p=P)
    ov = outT.ap().rearrange("(k p) n -> p k n", p=P)
    bv = bg.ap()

    silu = mybir.ActivationFunctionType.Silu

    with tile.TileContext(nc) as tc, ExitStack() as ctx:
        wpool = ctx.enter_context(tc.tile_pool(name="w", bufs=wb_bufs))
        hpool = ctx.enter_context(tc.tile_pool(name="h", bufs=h_gens))
        opool = ctx.enter_context(tc.tile_pool(name="o", bufs=o_gens))
        ppool = ctx.enter_context(
            tc.tile_pool(name="ps", bufs=ps_bufs, space="PSUM"))
        cpool = ctx.enter_context(tc.tile_pool(name="c", bufs=1))

        btile = cpool.tile([P, L * NS * KT], mybir.dt.float32)
        nc.scalar.dma_start(btile[:], bv[:, :])

        ms = list(reversed(range(KT)))
        ks = list(range(1, KT)) + [0]

        def body(_iv=None):
            frozen_w = {}
            for s in range(NS):
                tok, off = toks[s], int(offs[s])
                subs = _subs(tok, maxn)
                # one [P, 8*tok] activation tile per layer; column k*tok+i
                # holds token i of k-slice k
                hb = hpool.tile([P, KT * tok], ACT_DT, tag=f"A{tok}",
                                name="hin")
                if s == 0 and mode != "dma_only":
                    # split the x load so the k-outer matmuls stream behind
                    k0 = 0
                    for nk in x0_split:
                        nc.gpsimd.dma_start(
                            hb[:, k0 * tok:(k0 + nk) * tok],
                            xv[:, k0:k0 + nk, off:off + tok])
                        k0 += nk
                else:
                    nc.sync.dma_start(hb[:, :], xv[:, :, off:off + tok])
                for l in range(L):
                    # ---- weight bank DMA
                    if mode == "compute_only" and l in frozen_w:
                        wt = frozen_w[l]
                    else:
                        wt = wpool.tile([P, KT * D], W_DT, tag="wb",
                                        name=f"w{s}_{l}")
                        wsrc = wg.ap()[s, l].rearrange("(k p) m -> p k m",
                                                       p=P)
                        if (s == 0 and l == 0 and mode != "dma_only"
                                and not s0_gpsimd):
                            # per-k slices on SP: fast issue, PE streams
                            # behind them
                            for k in range(KT):
                                nc.sync.dma_start(
                                    wt[:, k * D:(k + 1) * D], wsrc[:, k, :])
                        elif s == 0 and l == 0 and mode != "dma_only":
                            # steady-state (reps>1): keep s0l0 off the SP
                            # queue so it isn't serialized behind the
                            # previous rep's output stores; per-k slices so
                            # the k-outer matmuls stream behind arrival
                            for k in range(KT):
                                nc.gpsimd.dma_start(
                                    wt[:, k * D:(k + 1) * D], wsrc[:, k, :])
                        elif w_split:
                            h = KT // 2
                            nc.sync.dma_start(wt[:, :h * D],
                                              wsrc[:, :h, :])
                            nc.gpsimd.dma_start(wt[:, h * D:],
                                                wsrc[:, h:, :])
                        else:
                            h = KT // 2
                            nc.gpsimd.dma_start(wt[:, :h * D],
                                                wsrc[:, :h, :])
                            nc.gpsimd.dma_start(wt[:, h * D:],
                                                wsrc[:, h:, :])
                        if mode == "compute_only":
                            frozen_w[l] = wt

                    if mode == "dma_only":
                        continue  # hb keeps the x data; out-DMA reads it

                    last = l == L - 1
                    if last:
                        ob = opool.tile([P, KT * tok], ACT_DT,
                                        tag=f"O{tok}", name="ob")
                    else:
                        ob = hpool.tile([P, KT * tok], ACT_DT,
                                        tag=f"A{tok}", name="hu")

                    def epilogue(m, si, ps):
                        soff, n = subs[si]
                        col = (l * NS + s) * KT + m
                        dst = ob[:, m * tok + soff:m * tok + soff + n]
                        if last:
                            # bias-add on DVE, bf16 out; ACT keeps running
                            # only Silu (no activation-table switches)
                            nc.vector.tensor_scalar_add(
                                dst, ps[:, :n], btile[:, col:col + 1])
                        else:
                            nc.scalar.activation(dst, ps[:, :n], silu,
                                                 bias=btile[:, col:col + 1])

                    if s == 0 and l == 0:
                        # k-outer: stream behind the first DMAs. PSUM holds
                        # 8 banks, so process m in groups of 8//nsub.
                        gsize = max(1, KT // len(subs))
                        for g0 in range(0, KT, gsize):
                            mg = ms[g0:g0 + gsize]
                            pss = {m: [ppool.tile([P, MAXN],
                                                  mybir.dt.float32,
                                                  tag="ps",
                                                  name=f"ps{m}_{si}")
                                       for si in range(len(subs))]
                                   for m in mg}
                            for j in range(KT):
                                for m in mg:
                                    for si, (soff, n) in enumerate(subs):
                                        nc.tensor.matmul(
                                            pss[m][si][:, :n],
                                            wt[:, j * D + m * P:
                                               j * D + (m + 1) * P],
                                            hb[:, j * tok + soff:
                                               j * tok + soff + n],
                                            start=(j == 0),
                                            stop=(j == KT - 1))
                            for m in mg:
                                for si in range(len(subs)):
                                    epilogue(m, si, pss[m][si])
                    else:
                        # m-outer, k rotated so the next layer consumes the
                        # previous layer's last-produced tile last
                        for m in ms:
                            pss = [ppool.tile([P, MAXN], mybir.dt.float32,
                                              tag="ps", name=f"psm{si}")
                                   for si in range(len(subs))]
                            for j, k in enumerate(ks):
                                # one_weight: timing diagnostic — reuse k=ks[0]
                                # weights for every k (wrong results)
                                kw = ks[0] if one_weight else k
                                lead = None
                                for si, (soff, n) in enumerate(subs):
                                    mm = nc.tensor.matmul(
                                        pss[si][:, :n],
                                        wt[:, kw * D + m * P:
                                           kw * D + (m + 1) * P],
                                        hb[:, k * tok + soff:
                                           k * tok + soff + n],
                                        start=(j == 0), stop=(j == KT - 1))
                                    if si == 0:
                                        lead = mm
                                    elif chain:
                                        # keep the pair adjacent on PE so
                                        # the LDWEIGHTS dedupe can fire
                                        add_dep_helper(
                                            mm.ins, lead.ins, sync=False,
                                            reason="ldw chain")
                            for si in range(len(subs)):
                                epilogue(m, si, pss[si])
                    hb = ob
                # ---- output DMA: one store for the whole slot; for the
                # final slot, stream per-m stores in production order so
                # the tail is one m-tile, not the whole slot
                if tail_stream and s == NS - 1 and mode != "dma_only":
                    for m in ms:
                        nc.sync.dma_start(ov[:, m, off:off + tok],
                                          hb[:, m * tok:(m + 1) * tok])
                else:
                    nc.sync.dma_start(ov[:, :, off:off + tok], hb[:, :])

        if reps == 1:
            body()
        else:
            with tc.For_i(0, reps, 1) as iv:
                body(iv)
    if chain:
        dedupe_ldweights(nc)
    if ldw_ahead:
        pipeline_ldweights(nc)
    elif selfload:
        fuse_ldweights(nc)
    nc.compile()
    return nc


# ---------------------------------------------------------------- host glue

def prepare_in_maps(x, cat_ids, Ws, bs, profile, cores):
    x = np.asarray(x)
    cat_ids = np.asarray(cat_ids).astype(np.int64)
    sorder = _slot_order(profile)
    NS = len(profile)
    in_maps = []
    for c in range(NCORES):
        slots = [cores[c][pos] for pos in sorder]
        samp = [i for sl in slots for i in sl]
        xs = np.asarray(x[samp], dtype=np.float32)
        xTc = np.ascontiguousarray(xs.reshape(len(samp) * TOK, D).T)
        cats = [int(cat_ids[sl[0]]) for sl in slots]
        wgc = np.stack([np.stack([Ws[l][cat] for l in range(L)])
                        for cat in cats])
        # [L, NS, D] -> [P, L*NS*KT] with element [p, (l*NS+s)*KT+m]
        # = b_l[cat_s][m*P+p]
        bgc = np.stack([np.stack([bs[l][cat] for cat in cats])
                        for l in range(L)])
        bgc = bgc.reshape(L, NS, KT, P).transpose(3, 0, 1, 2).reshape(
            P, L * NS * KT)
        in_maps.append({
            "xT": xTc.astype(ACT_NP),
            "wg": np.ascontiguousarray(wgc).astype(W_NP),
            "bg": np.ascontiguousarray(bgc).astype(np.float32),
        })
    return in_maps


def finish_output(results, profile, cores, B):
    sorder = _slot_order(profile)
    out = np.empty((B, TOK, D), np.float32)
    for c in range(NCORES):
        slots = [cores[c][pos] for pos in sorder]
        samp = [i for sl in slots for i in sl]
        outTc = np.asarray(results[c]["outT"], dtype=np.float32)
        out[samp] = outTc.T.reshape(len(samp), TOK, D)
    return out


def kernel(x, cat_ids, W1, b1, W2, b2, W3, b3, W4, b4):
    global LAST_RESULT
    cat_ids = np.asarray(cat_ids).astype(np.int64)
    Ws = [np.asarray(w, dtype=np.float32) for w in (W1, W2, W3, W4)]
    bs = [np.asarray(b, dtype=np.float32) for b in (b1, b2, b3, b4)]
    x = np.asarray(x, dtype=np.float32)
    B = x.shape[0]

    profile, cores = plan(cat_ids)
    in_maps = prepare_in_maps(x, cat_ids, Ws, bs, profile, cores)

    if profile not in _PROGRAM_CACHE:
        _PROGRAM_CACHE[profile] = build_program(profile)
    nc = _PROGRAM_CACHE[profile]

    res = run_bass_kernel_spmd(nc, in_maps, list(range(NCORES)))
    LAST_RESULT = res
    return finish_output(res.results, profile, cores, B)

